# revision 26
# baseline (speedup 1.0000x reference)
"""Two-layer GAT (PyG GATConv, heads=3, concat=False/mean) on 8 trn2 NeuronCores.

Strategy (1D dest-partitioning, v3):
  - dests sharded 6250/core; each core owns all edges INTO its dests.
  - dense projection H = X @ Wfold computed replicated per core into a DRAM
    table with partition-major row order (row r = p*T + t holds the node at
    xT column t*128+p), so the dense-phase stores are 128 large descriptors.
  - table row (256 bf16): [h 0:192 | ones 192:195 | a_src 195:198 | a_dst 198:201 | pad]
  - per-edge rows fetched with dma_gather (int16 idx; two index windows A/B).
    Blocks are processed in super-blocks of G=2 dest-blocks -> 2 gather calls
    per super-block.
  - attention: ad broadcast edge-wise via ST one-hot matmuls (PE); e=as+ad,
    m=lrelu(e) and p=exp(m) expanded to a contiguous [P, C*256] buffer by the
    Scalar engine; ONE contiguous bf16 DVE multiply forms [p*h | p]; per-block
    one-hot S matmuls accumulate numerators+denominators in PSUM.
  - S/ST stored interleaved per chunk in one partition-major u8 DRAM tensor,
    loaded (with u8->bf16 cast) in one DMA per super-block.
  - two launches of ONE compiled program (layer 2 input relayed via host).
"""
import sys

if '/opt/trn_rl_repo' not in sys.path:
    sys.path.insert(0, '/opt/trn_rl_repo')

import os
import types

import numpy as np
import ml_dtypes

import concourse.bass as bass
import concourse.bacc as bacc
import concourse.tile as tile
from concourse import mybir
from concourse.bass_utils import run_bass_kernel_spmd

timed_ns = None


def _try_install_profile_hook():
    try:
        if 'antenv.axon_hooks' in sys.modules:
            return True
        if '/root/.axon_site' not in sys.path:
            sys.path.insert(0, '/root/.axon_site')
        from trn_agent_boot.trn_boot import _ntff_profile_via_ctypes
        hook = _ntff_profile_via_ctypes('/opt/axon/libaxon_pjrt.so')
        mod = types.ModuleType('antenv.axon_hooks')
        mod.get_axon_ntff_profile_hook = lambda: hook
        mod.set_axon_ntff_profile_hook = lambda h: None
        import antenv
        sys.modules['antenv.axon_hooks'] = mod
        antenv.axon_hooks = mod
        from concourse import bass_utils
        bass_utils.upload_artifacts = lambda tmpdir: tmpdir
        return True
    except Exception:
        return False

BF16 = ml_dtypes.bfloat16

N = 50000
IN_F = 128
HID = 64
HEADS = 3
NEG = 0.2
W = 8                 # cores
NLOC = N // W         # 6250 dests per core
P = 128
NBLK = (NLOC + P - 1) // P          # 49 dest blocks per core
ROW = 256                            # table row elems (bf16, 512B)
T = 391                              # node tiles
NPAD = P * T                         # 50048
HALF_A = 32768                       # gather window A = rows [0, 32768)
G = 2                                # dest blocks per super-block
NSB = (NBLK + G - 1) // G
MAXCH = int(os.environ.get('GAT_MAXCH', '8'))   # chunks per dma_gather call


def _build_structure(edge_index):
    """Host-side: per-core edge chunking, index & one-hot tensors."""
    src = np.asarray(edge_index[0]).astype(np.int64)
    dst = np.asarray(edge_index[1]).astype(np.int64)
    loop = np.arange(N, dtype=np.int64)
    s_all = np.concatenate([src, loop])
    d_all = np.concatenate([dst, loop])

    # per-core node->row map: own dests first (rows 0..NLOC-1), rest after.
    rowmap = np.empty((W, N), np.int64)
    for c in range(W):
        own = np.arange(c * NLOC, (c + 1) * NLOC)
        others = np.concatenate([np.arange(0, c * NLOC), np.arange((c + 1) * NLOC, N)])
        rowmap[c, own] = np.arange(NLOC)
        rowmap[c, others] = NLOC + np.arange(N - NLOC)

    core_of = d_all // NLOC
    blk_edges = [[None] * NBLK for _ in range(W)]
    kA = np.zeros((W, NBLK), np.int64)
    kB = np.zeros((W, NBLK), np.int64)
    for c in range(W):
        sel = core_of == c
        es = s_all[sel]
        ed = d_all[sel] - c * NLOC
        order = np.argsort(ed, kind='stable')
        es, ed = es[order], ed[order]
        erow = rowmap[c, es]
        blk = ed // P
        for b in range(NBLK):
            m = blk == b
            er, dl = erow[m], ed[m] - b * P
            isA = er < HALF_A
            eA_r, eA_d = er[isA], dl[isA]
            eB_r, eB_d = er[~isA] - HALF_A, dl[~isA]
            oA = np.argsort(eA_r, kind='stable')
            oB = np.argsort(eB_r, kind='stable')
            blk_edges[c][b] = (eA_r[oA], eA_d[oA], eB_r[oB], eB_d[oB])
            kA[c, b] = (len(eA_r) + P - 1) // P
            kB[c, b] = (len(eB_r) + P - 1) // P
    kA_u = np.maximum(kA.max(axis=0), 1)
    kB_u = np.maximum(kB.max(axis=0), 1)

    # super-block structure: sb covers blocks [G*sb, min(G*sb+G, NBLK))
    # chunk order within sb: A-chunks blk0, A-chunks blk1, B-chunks blk0, B...
    sb_blocks = [list(range(G * s, min(G * s + G, NBLK))) for s in range(NSB)]
    sb_off = []          # global chunk offset of each super-block
    blk_chunk_pos = {}   # block -> list of global chunk positions (in order)
    coff = 0
    for s, blks in enumerate(sb_blocks):
        sb_off.append(coff)
        o = coff
        for b in blks:
            blk_chunk_pos[b] = list(range(o, o + int(kA_u[b])))
            o += int(kA_u[b])
        for b in blks:
            blk_chunk_pos[b] += list(range(o, o + int(kB_u[b])))
            o += int(kB_u[b])
        coff = o
    C_total = coff

    idx_cols = C_total * 8    # 8 idx16 cols per chunk
    out = {
        'kA': kA_u, 'kB': kB_u, 'sb_blocks': sb_blocks, 'sb_off': sb_off,
        'blk_chunk_pos': blk_chunk_pos, 'C_total': C_total, 'rowmap': rowmap,
        'idx16': np.zeros((W, P, idx_cols), np.int16),
        'sst': np.zeros((W, P, C_total * 2 * P), np.uint8),
    }
    for c in range(W):
        flat_rows = np.zeros((C_total, P), np.int64)
        flat_dl = np.full((C_total, P), -1, np.int64)
        for b in range(NBLK):
            eA_r, eA_d, eB_r, eB_d = blk_edges[c][b]
            pos = blk_chunk_pos[b]
            nA = int(kA_u[b])
            ra = np.zeros(nA * P, np.int64)
            da = np.full(nA * P, -1, np.int64)
            ra[:len(eA_r)] = eA_r
            da[:len(eA_d)] = eA_d
            for j in range(nA):
                flat_rows[pos[j]] = ra[j * P:(j + 1) * P]
                flat_dl[pos[j]] = da[j * P:(j + 1) * P]
            nB = int(kB_u[b])
            rb = np.zeros(nB * P, np.int64)
            db = np.full(nB * P, -1, np.int64)
            rb[:len(eB_r)] = eB_r
            db[:len(eB_d)] = eB_d
            for j in range(nB):
                flat_rows[pos[nA + j]] = rb[j * P:(j + 1) * P]
                flat_dl[pos[nA + j]] = db[j * P:(j + 1) * P]

        # S/ST interleaved, partition-major: for chunk k,
        #   sst[p, k*256 + d]      = S[edge p -> dest d]   (one-hot row of edge p)
        #   sst[p, k*256 + 128 + e] = ST[dest p <- edge e]
        sst = out['sst'][c]
        ch = np.repeat(np.arange(C_total), P)
        ee = np.tile(np.arange(P), C_total)
        dl = flat_dl.reshape(-1)
        v = dl >= 0
        sst[ee[v], ch[v] * 2 * P + dl[v]] = 1
        sst[dl[v], ch[v] * 2 * P + P + ee[v]] = 1

        # idx16: per super-block, two calls (A then B); within a call the
        # chunk-major flat rows are wrapped [16, nch*8] and tiled to 128 parts.
        for s, blks in enumerate(sb_blocks):
            nA = sum(int(kA_u[b]) for b in blks)
            nB = sum(int(kB_u[b]) for b in blks)
            co = sb_off[s]
            for (o, nch) in ((0, nA), (nA, nB)):
                flat = flat_rows[co + o:co + o + nch].reshape(-1)
                wrapped = np.zeros((16, nch * 8), np.int16)
                i = np.arange(nch * P)
                wrapped[i % 16, i // 16] = flat.astype(np.int16)
                col = (co + o) * 8
                out['idx16'][c, :, col:col + nch * 8] = np.tile(wrapped, (8, 1))
    return out


def _fold_w(Wm, a_src, a_dst):
    """wf [128, 256]: cols 0:192 = W (heads), 195:198 = as-proj, 198:201 = ad-proj.
    Cols 192:195 stay 0 (overwritten with 1.0 in the staged table)."""
    in_f = Wm.shape[0]
    Wf = np.zeros((P, ROW), np.float32)
    Wf[:in_f, 0:192] = Wm
    for h in range(HEADS):
        Wf[:in_f, 195 + h] = Wm[:, h * HID:(h + 1) * HID] @ a_src[h]
        Wf[:in_f, 198 + h] = Wm[:, h * HID:(h + 1) * HID] @ a_dst[h]
    return Wf.astype(BF16)


def _build_nc(st):
    kA, kB = st['kA'], st['kB']
    sb_blocks, sb_off = st['sb_blocks'], st['sb_off']
    blk_chunk_pos, C_total = st['blk_chunk_pos'], st['C_total']
    idx_cols = st['idx16'].shape[2]

    nc = bacc.Bacc("TRN2", target_bir_lowering=False, debug=False,
                   num_swdge_queues=4)
    xT_in = nc.declare_dram_parameter("xT", [P, NPAD], mybir.dt.bfloat16, isOutput=False)
    wf_in = nc.declare_dram_parameter("wf", [P, ROW + 4], mybir.dt.bfloat16, isOutput=False)
    sst_in = nc.declare_dram_parameter("sst_u8", [P, C_total * 2 * P], mybir.dt.uint8, isOutput=False)
    idx_in = nc.declare_dram_parameter("idx16", [P, idx_cols], mybir.dt.int16, isOutput=False)
    out_raw = nc.declare_dram_parameter("out_raw", [NLOC, HID], mybir.dt.float32, isOutput=True)

    table = nc.dram_tensor("table", [NPAD, ROW], mybir.dt.bfloat16)
    table3 = table[:].rearrange("(p t) r -> p t r", t=T)

    DT = mybir.dt.bfloat16
    F32 = mybir.dt.float32
    DGRP = 16

    with tile.TileContext(nc) as tc:
        with (
            tc.tile_pool(name="const", bufs=1) as cpool,
            tc.tile_pool(name="dense", bufs=2) as dpool,
            tc.tile_pool(name="dstage", bufs=2) as spool_d,
            tc.tile_pool(name="dpsum", bufs=3, space="PSUM") as dpsum,
            tc.tile_pool(name="gath", bufs=3) as gpool,
            tc.tile_pool(name="sst", bufs=3) as sstpool,
            tc.tile_pool(name="pexp", bufs=2) as ppool,
            tc.tile_pool(name="blk", bufs=3) as bpool,
            tc.tile_pool(name="apsum", bufs=3, space="PSUM") as apsum,
            tc.tile_pool(name="adpsum", bufs=2, space="PSUM") as adpsum,
        ):
            wf_t = cpool.tile([P, ROW + 4], DT)
            nc.sync.dma_start(out=wf_t[:], in_=wf_in[:])
            ones3 = wf_t[:, ROW:ROW + 3]
            idx_t = cpool.tile([P, idx_cols], mybir.dt.int16)
            nc.sync.dma_start(out=idx_t[:], in_=idx_in[:])

            # ---- dense phase: table[p*T+t, :] = (xT[:, t*128+p]).T @ wf ----
            for g0 in range(0, T, DGRP):
                g1 = min(g0 + DGRP, T)
                ng = g1 - g0
                xg = dpool.tile([P, DGRP * P], DT, tag="xg")
                nc.sync.dma_start(out=xg[:, :ng * P], in_=xT_in[:, g0 * P:g1 * P])
                hstage = spool_d.tile([P, DGRP * ROW], DT, tag="hstage")
                hstage3 = hstage[:].rearrange("p (t r) -> p t r", r=ROW)
                for i, t0 in enumerate(range(g0, g1, 2)):
                    npair = min(2, g1 - t0)
                    ps = dpsum.tile([P, 2 * ROW], F32)
                    ps3 = ps[:].rearrange("p (t r) -> p t r", r=ROW)
                    for u in range(npair):
                        nc.tensor.matmul(out=ps3[:, u, 0:204],
                                         lhsT=xg[:, (t0 - g0 + u) * P:(t0 - g0 + u + 1) * P],
                                         rhs=wf_t[:, 0:204], start=True, stop=True)
                    dst_sl = hstage3[:, t0 - g0:t0 - g0 + npair, 0:204]
                    if i % 2 == 0:
                        nc.vector.tensor_copy(out=dst_sl, in_=ps3[:, 0:npair, 0:204])
                    else:
                        nc.scalar.activation(dst_sl, ps3[:, 0:npair, 0:204],
                                             mybir.ActivationFunctionType.Copy)
                # overwrite cols 192:195 of each staged row with 1.0
                nc.vector.tensor_copy(
                    out=hstage[:, :ng * ROW].rearrange("p (t r) -> p t r", r=ROW)[:, :, 192:195],
                    in_=ones3.rearrange("p (o r) -> p o r", o=1).broadcast_to([P, ng, 3]),
                )
                nc.sync.dma_start(
                    out=table3[:, g0:g1, :],
                    in_=hstage[:, :ng * ROW].rearrange("p (t r) -> p t r", r=ROW),
                )

            # ---- aggregation phase ----
            for s, blks in enumerate(sb_blocks):
                co = sb_off[s]
                nA = sum(int(kA[b]) for b in blks)
                nB = sum(int(kB[b]) for b in blks)
                ck = nA + nB

                hg = gpool.tile([P, ck * ROW], DT, tag="hg")
                hg3 = hg[:].rearrange("p (k r) -> p k r", r=ROW)
                qn = 0
                for (half, o0, nch0) in ((0, 0, nA), (1, nA, nB)):
                    base = HALF_A * half
                    top = HALF_A if half == 0 else NPAD - HALF_A
                    for o in range(o0, o0 + nch0, MAXCH):
                        nch = min(MAXCH, o0 + nch0 - o)
                        col = (co + o) * 8
                        nc.gpsimd.dma_gather(
                            out_ap=hg3[:, o:o + nch, :],
                            in_ap=table[base:base + top, :],
                            idxs_ap=idx_t[:, col:col + nch * 8],
                            num_idxs=nch * P,
                            num_idxs_reg=nch * P,
                            elem_size=ROW,
                            queue_num=(2 * s + qn) % 4,
                        )
                        qn += 1

                sst_t = sstpool.tile([P, ck * 2 * P], DT, tag="sst")
                nc.gpsimd.dma_start(
                    out=sst_t[:],
                    in_=sst_in[:, co * 2 * P:(co + ck) * 2 * P],
                )

                # dest-attention per block -> broadcast to edges via ST matmuls
                ad_ps = adpsum.tile([P, ((ck * 3 + 15) // 16) * 16], F32)
                for b in blks:
                    adB = bpool.tile([P, 4], DT, tag="adB")
                    nc.sync.dma_start(out=adB[:], in_=table[b * P:(b + 1) * P, 198:202])
                    for j in blk_chunk_pos[b]:
                        jj = j - co
                        nc.tensor.matmul(out=ad_ps[:, jj * 3:(jj + 1) * 3],
                                         lhsT=sst_t[:, jj * 2 * P + P:(jj + 1) * 2 * P],
                                         rhs=adB[:, 0:3], start=True, stop=True)

                # e = as + ad ; m = lrelu(e) ; pexp = exp(m) expanded
                e_t = bpool.tile([P, ck * 3], F32, tag="e")
                nc.vector.tensor_tensor(
                    out=e_t[:].rearrange("p (k t) -> p k t", t=3),
                    in0=hg3[:, :, 195:198],
                    in1=ad_ps[:, 0:ck * 3].rearrange("p (k t) -> p k t", t=3),
                    op=mybir.AluOpType.add)
                lr_t = bpool.tile([P, ck * 3], F32, tag="lr")
                nc.scalar.activation(lr_t[:], e_t[:],
                                     mybir.ActivationFunctionType.Copy, scale=NEG)
                m_t = bpool.tile([P, ck * 3], F32, tag="m")
                nc.vector.tensor_tensor(out=m_t[:], in0=lr_t[:], in1=e_t[:],
                                        op=mybir.AluOpType.max)
                m3 = m_t[:].rearrange("p (k t) -> p k t", t=3)
                pexp = ppool.tile([P, ck * 204], DT, tag="pexp")
                pexp3 = pexp[:].rearrange("p (k r) -> p k r", r=204)
                for h in range(HEADS):
                    nc.scalar.activation(
                        pexp3[:, :, h * HID:(h + 1) * HID],
                        m3[:, :, h:h + 1].broadcast_to([P, ck, HID]),
                        mybir.ActivationFunctionType.Exp)
                nc.scalar.activation(pexp3[:, :, 192:195], m3[:],
                                     mybir.ActivationFunctionType.Exp)

                # hp = hg * pexp  (bf16, in place)
                nc.vector.tensor_tensor(out=hg3[:, :, 0:204], in0=hg3[:, :, 0:204],
                                        in1=pexp3[:, :, 0:204],
                                        op=mybir.AluOpType.mult)

                # accumulate per block: acc[d, 0:195] += S_j.T @ hp_j[:, 0:195]
                for b in blks:
                    pos = blk_chunk_pos[b]
                    ndest = min(P, NLOC - b * P)
                    acc = apsum.tile([P, 208], F32)
                    for i, j in enumerate(pos):
                        jj = j - co
                        nc.tensor.matmul(out=acc[:, 0:195],
                                         lhsT=sst_t[:, jj * 2 * P:jj * 2 * P + P],
                                         rhs=hg3[:, jj, 0:195],
                                         start=(i == 0), stop=(i == len(pos) - 1))

                    # epilogue: out = mean_h(num_h / den_h)
                    # table "ones" hold 3.0, so acc[:,192:195] = 3*sum(p): the
                    # head-mean 1/3 is already folded into the reciprocal.
                    rec = bpool.tile([P, 3], F32, tag="rec")
                    nc.vector.reciprocal(out=rec[:], in_=acc[:, 192:195])
                    o3 = bpool.tile([P, 3 * HID], F32, tag="o3")
                    nc.vector.tensor_tensor(
                        out=o3[:].rearrange("p (h f) -> p h f", f=HID),
                        in0=acc[:, 0:3 * HID].rearrange("p (h f) -> p h f", f=HID),
                        in1=rec[:].rearrange("p (h o) -> p h o", o=1).broadcast_to([P, 3, HID]),
                        op=mybir.AluOpType.mult)
                    o_raw = bpool.tile([P, HID], F32, tag="oraw")
                    nc.vector.tensor_tensor(out=o_raw[:], in0=o3[:, 0:HID],
                                            in1=o3[:, HID:2 * HID],
                                            op=mybir.AluOpType.add)
                    nc.vector.tensor_tensor(out=o_raw[:], in0=o_raw[:],
                                            in1=o3[:, 2 * HID:3 * HID],
                                            op=mybir.AluOpType.add)
                    nc.sync.dma_start(out=out_raw[b * P:b * P + ndest, :],
                                      in_=o_raw[:ndest, :])

    nc.compile()
    return nc


def kernel(**inputs):
    x = np.asarray(inputs['x'], np.float32)
    edge_index = np.asarray(inputs['edge_index'])
    st = _build_structure(edge_index)
    nc = _build_nc(st)

    rowmap = st['rowmap']

    def xT_for(core, feats):
        in_f = feats.shape[1]
        xrow = np.zeros((NPAD, P), BF16)           # by table row
        xrow[rowmap[core], :in_f] = feats.astype(BF16)
        # table row r = p*T + t lives at xT column t*128+p
        xcol = xrow.reshape(P, T, P).transpose(1, 0, 2).reshape(NPAD, P)
        return np.ascontiguousarray(xcol.T)

    def run_layer(feats, Wm, a_src, a_dst):
        wf = _fold_w(np.asarray(Wm, np.float32),
                     np.asarray(a_src, np.float32), np.asarray(a_dst, np.float32))
        wfp = np.zeros((P, ROW + 4), BF16)
        wfp[:, :ROW] = wf
        wfp[:, ROW:ROW + 3] = np.float32(3.0)
        in_maps = []
        for c in range(W):
            in_maps.append({
                'xT': xT_for(c, feats),
                'wf': wfp,
                'sst_u8': st['sst'][c],
                'idx16': st['idx16'][c],
            })
        trace = os.environ.get('GAT_PROFILE') == '1' and _try_install_profile_hook()
        res = run_bass_kernel_spmd(nc, in_maps, core_ids=list(range(W)), trace=trace)
        global timed_ns
        if trace and res.exec_time_ns:
            timed_ns = (timed_ns or 0) + res.exec_time_ns
        raw = np.concatenate([res.results[c]['out_raw'] for c in range(W)], axis=0)
        return raw

    raw1 = run_layer(x, inputs['W1'], inputs['att_src1'], inputs['att_dst1'])
    if os.environ.get('GAT_DEBUG_SAVE'):
        np.save('/tmp/gat_raw1.npy', raw1)
    h1 = np.maximum(raw1 + np.asarray(inputs['bias1'], np.float32)[None, :], 0.0)
    out = run_layer(h1, inputs['W2'], inputs['att_src2'], inputs['att_dst2'])
    out = out + np.asarray(inputs['bias2'], np.float32)[None, :]
    return out.astype(np.float32)
